# revision 44
# baseline (speedup 1.0000x reference)
"""Trainium2 Bass kernel: standard multi-head attention (B=2, S=2048, H=16, D=128, fp32).

Sharding: head-parallel across 8 NeuronCores (2 heads per core, both batches),
zero cross-core communication.

Host side (part of sharding): per core, Q and K head-slices are transposed to
[d, s] and cast to fp16; V is cast to fp16 and augmented with a ones column
(so the PV matmul accumulates the softmax denominator for free); the key
padding mask becomes additive biases laid out per k-tile (one set for the ACT
exp path, one pre-folded into the Schraudolph constant for the DVE path).

Per-core device program, per (batch, head) unit:
  - DMA qT, kT, V_aug (fp16) straight into SBUF.
  - For each 1024-wide q-chunk, 16 steps (one per k-tile):
      S^T[k, q] = (K Q^T) via PE matmul (fp16 in, fp32 PSUM).
      P^T = exp(scale * S^T + bias) -> fp16 SBUF, split across two engines:
        * 11/16 k-tiles on ACT (table exp),
        * 5/16 k-tiles on DVE via a one-instruction Schraudolph exp:
          int16 bits = round(S^T * (1024*log2e*scale) + (15360 - C + bias'))
          bit-cast to fp16. C=60 centers the ripple (~1.7% rms), keeping
          total output error ~9e-3, well under the 2e-2 gate.
    PV for the previous chunk is interleaved: per q-tile a single 16-matmul
    PSUM accumulation group over all k-tiles (d'=128 is the denominator
    column), then one DVE reciprocal + tensor_scalar multiply normalizes
    straight out of PSUM into the chunk's output staging tile.
  - One output DMA per chunk.

PE p-state is pre-warmed with dummy matmuls during the initial DMA fill so
real matmuls start at full clock.  All accumulation fp32; matmul inputs fp16.
"""

import numpy as np

B, S, H, D = 2, 2048, 16, 128
NCORES = 8
H2 = H // NCORES          # heads per core
KTILES = S // 128         # 16
VW = D + 2                # V_aug row width: 128 d + ones col + pad
QCHUNK = 1024
NQC = S // QCHUNK         # 2
QT = QCHUNK // 128        # 8 q-tiles per chunk
SCALE = 1.0 / float(np.sqrt(D))
MASK_NEG = -30.0

# Schraudolph fp16 exp constants (DVE path)
LOG2E = 1.4426950408889634
EXP_A = float(1024.0 * LOG2E * SCALE)   # multiplies the raw (unscaled) score
EXP_C = 60.0                            # ripple-centering correction
EXP_B0 = float(15360.0 - EXP_C)         # fp16 exponent-bias term
DVE_STEPS = (2, 5, 8, 11, 14)           # k-tiles handled by the DVE exp
# First chunk has no PV interleave, so QK outpaces the exp drain; split the
# exp 8/8 between ACT and DVE there (GpSimd can't read PSUM).
DVE_STEPS_C0 = (1, 3, 5, 7, 9, 11, 13, 15)
POOL_STEPS_C0 = ()

N_WARM = 40                             # PE p-state warm-up matmuls

_CACHE = {}


def _build_program():
    from contextlib import ExitStack

    import concourse.tile as tile
    from concourse import bacc, mybir

    f32 = mybir.dt.float32
    f16 = mybir.dt.float16
    i16 = mybir.dt.int16

    nc = bacc.Bacc("TRN2", target_bir_lowering=False, debug=False, num_devices=NCORES)
    qt_d = nc.dram_tensor("qt", [B, H2, D, S], f16, kind="ExternalInput").ap()
    kt_d = nc.dram_tensor("kt", [B, H2, D, S], f16, kind="ExternalInput").ap()
    va_d = nc.dram_tensor("va", [B, H2, 128, KTILES, VW], f16, kind="ExternalInput").ap()
    # biases[..., 0] = ACT exp mask bias; biases[..., 1] = DVE Schraudolph const
    biases_d = nc.dram_tensor("biases", [B, 128, KTILES, 2], f32, kind="ExternalInput").ap()
    o_d = nc.dram_tensor("o", [B, S, H2, D], f32, kind="ExternalOutput").ap()

    EXP = mybir.ActivationFunctionType.Exp
    MULT = mybir.AluOpType.mult
    ADD = mybir.AluOpType.add

    with tile.TileContext(nc) as tc, ExitStack() as ctx:
        tpool = ctx.enter_context(tc.tile_pool(name="tpool", bufs=2))
        vpool = ctx.enter_context(tc.tile_pool(name="vpool", bufs=2))
        bpool = ctx.enter_context(tc.tile_pool(name="bpool", bufs=1))
        ppool = ctx.enter_context(tc.tile_pool(name="ppool", bufs=34))
        opool = ctx.enter_context(tc.tile_pool(name="opool", bufs=2))
        rpool = ctx.enter_context(tc.tile_pool(name="rpool", bufs=4))
        wpool = ctx.enter_context(tc.tile_pool(name="wpool", bufs=1))
        st_ps = ctx.enter_context(tc.tile_pool(name="st_ps", bufs=3, space="PSUM"))
        o_ps = ctx.enter_context(tc.tile_pool(name="o_ps", bufs=2, space="PSUM"))

        # mask biases for both batches, [B, 128, KTILES, 2] (partition-major)
        bias_sb = bpool.tile([128, B, KTILES, 2], f32, name="bias_sb", tag="bias")

        units = [(b, h) for b in range(B) for h in range(H2)]
        chunks = [(u, qc) for u in range(len(units)) for qc in range(NQC)]

        def prep(u, fine=False):
            """DMA the unit's pre-transposed fp16 tensors into SBUF.

            fine=True (first unit only) orders/splits the loads so the first
            QK matmul's operands land as early as possible."""
            b, h = units[u]
            qt = tpool.tile([128, S], f16, name="qt_sb", tag="qt")
            kt = tpool.tile([128, S], f16, name="kt_sb", tag="kt")
            va = vpool.tile([128, KTILES, VW], f16, name="va_sb", tag="va")
            if fine:
                # overlap the critical first loads across the two HWDGE
                # queues: sync's FIFO leads with qt-lo, scalar's with kt-lo
                nc.sync.dma_start(qt[:, 0:512], qt_d[b, h, :, 0:512])
                nc.sync.dma_start(qt[:, 512:QCHUNK], qt_d[b, h, :, 512:QCHUNK])
                nc.scalar.dma_start(bias_sb[:], biases_d.rearrange("b p t e -> p b t e"))
                nc.scalar.dma_start(kt[:, 0:QCHUNK], kt_d[b, h, :, 0:QCHUNK])
                nc.sync.dma_start(kt[:, QCHUNK:S], kt_d[b, h, :, QCHUNK:S])
                nc.scalar.dma_start(va[:, 0 : KTILES // 2, :], va_d[b, h, :, 0 : KTILES // 2, :])
                nc.sync.dma_start(va[:, KTILES // 2 :, :], va_d[b, h, :, KTILES // 2 :, :])
                nc.sync.dma_start(qt[:, QCHUNK:S], qt_d[b, h, :, QCHUNK:S])
            else:
                nc.sync.dma_start(kt[:], kt_d[b, h])
                nc.sync.dma_start(qt[:], qt_d[b, h])
                nc.scalar.dma_start(va[:], va_d[b, h])
            return {"q": qt, "k": kt, "v": va}

        unit_tiles = {0: prep(0, fine=True)}

        # warm up the ACT exp table before any data arrives (table load
        # ~1.3us); emitted after prep so the scalar queue's DMAs issue first
        warm = rpool.tile([128, 1], f32, name="warm", tag="warm")
        nc.vector.memset(warm[:], 0.0)
        nc.scalar.activation(warm[:], warm[:], EXP, bias=0.0, scale=1.0)

        # PE p-state warm-up: dummy matmuls on zeroed SBUF during DMA fill.
        wz = wpool.tile([128, 512], f16, name="wz", tag="wz")
        nc.vector.memset(wz[:], 0.0)
        wacc = o_ps.tile([128, D + 1], f32, name="wacc", tag="oacc")
        for _ in range(N_WARM):
            nc.tensor.matmul(
                wacc[:], lhsT=wz[:, 0:128], rhs=wz[:, 0 : D + 1],
                start=True, stop=True,
            )

        state = {}

        def emit_s(c, j):
            """QK^T matmuls for k-tile j, then exp on ACT or DVE."""
            u, qc = chunks[c]
            b, h = units[u]
            tl = unit_tiles[u]
            q0 = qc * QCHUNK
            st = st_ps.tile([128, QCHUNK], f32, name="st", tag="st")
            for half in range(QCHUNK // 512):
                nc.tensor.matmul(
                    st[:, half * 512 : (half + 1) * 512],
                    lhsT=tl["k"][:, j * 128 : (j + 1) * 128],
                    rhs=tl["q"][:, q0 + half * 512 : q0 + (half + 1) * 512],
                    start=True,
                    stop=True,
                )
            pt = ppool.tile([128, QCHUNK], f16, name="pt", tag="pt")
            dve_steps = DVE_STEPS_C0 if c == 0 else DVE_STEPS
            pool_steps = POOL_STEPS_C0 if c == 0 else ()
            if j in dve_steps or j in pool_steps:
                eng = nc.vector if j in dve_steps else nc.gpsimd
                eng.tensor_scalar(
                    pt[:].bitcast(mybir.dt.int16),
                    st[:],
                    EXP_A,
                    bias_sb[:, b, j, 1:2],
                    MULT,
                    ADD,
                )
            else:
                nc.scalar.activation(
                    pt[:], st[:], EXP, bias=bias_sb[:, b, j, 0:1], scale=SCALE
                )
            state[c]["pt"].append(pt)

        KH = KTILES // 2

        def emit_pv_step(c, step):
            """8 PV matmuls (half a q-tile's accumulation group) + finish."""
            stt = state[c]
            t, half = step // 2, step % 2
            if half == 0:
                stt["oacc"] = o_ps.tile([128, D + 1], f32, name="oacc", tag="oacc")
            oacc = stt["oacc"]
            for j in range(half * KH, half * KH + KH):
                nc.tensor.matmul(
                    oacc[:],
                    lhsT=stt["pt"][j][:, t * 128 : (t + 1) * 128],
                    rhs=stt["v16"][:, j, 0 : D + 1],
                    start=(j == 0),
                    stop=(j == KTILES - 1),
                )
            if half == 1:
                rec = rpool.tile([128, 1], f32, name="rec", tag="rec")
                nc.vector.reciprocal(rec[:], oacc[:, D : D + 1])
                nc.vector.tensor_scalar_mul(stt["osb"][:, t, :], oacc[:, 0:D], rec[:])
                # store in two half-chunk DMAs so the tail overlaps the drain;
                # the very last half goes out per-qtile from the vector engine
                # (no sync round-trip) to shorten the post-drain critical path
                u, qc = chunks[c]
                b, h = units[u]
                if c == nchunks - 1 and t >= QT // 2:
                    nc.scalar.dma_start(
                        o_d[b, :, h, :].rearrange(
                            "(cc t p) d -> cc t p d", cc=NQC, t=QT, p=128
                        )[qc, t],
                        stt["osb"][:, t, :],
                    )
                    if t == QT - 1:
                        del state[c]
                elif t == QT // 2 - 1 or t == QT - 1:
                    hq = 0 if t < QT // 2 else 1
                    sl = slice(hq * (QT // 2), (hq + 1) * (QT // 2))
                    nc.sync.dma_start(
                        o_d[b, :, h, :].rearrange(
                            "(cc hh t p) d -> cc hh p t d",
                            cc=NQC, hh=2, t=QT // 2, p=128,
                        )[qc, hq],
                        stt["osb"][:, sl, :],
                    )
                    if t == QT - 1:
                        del state[c]

        nchunks = len(chunks)
        for c in range(nchunks + 1):
            if c < nchunks:
                u, qc = chunks[c]
                state[c] = {
                    "pt": [],
                    "v16": unit_tiles[u]["v"],
                    "osb": opool.tile([128, QT, D], f32, name="osb", tag="osb"),
                }
                # prefetch next unit's tensors one chunk ahead
                if qc == NQC - 1 and u + 1 < len(units):
                    unit_tiles[u + 1] = prep(u + 1)
            for step in range(KTILES):
                if c < nchunks:
                    emit_s(c, step)
                if c > 0:
                    emit_pv_step(c - 1, step)
            if c == nchunks:
                break

    nc.compile()
    return nc


def _get_program():
    if "nc" not in _CACHE:
        _CACHE["nc"] = _build_program()
    return _CACHE["nc"]


def make_core_inputs(q, k, v, key_padding_mask):
    """Shard full inputs into per-core input maps (host side).

    Layout work done here (part of sharding): head-slice, transpose Q/K to
    [d, s], cast to fp16, build ones-augmented V, mask -> additive biases.
    """
    q = np.asarray(q, dtype=np.float32)
    k = np.asarray(k, dtype=np.float32)
    v = np.asarray(v, dtype=np.float32)

    mb = np.where(key_padding_mask, 0.0, MASK_NEG).astype(np.float32)
    # mb[b, s] with s = 128*t + p  ->  [B, 128(p), KTILES(t)]
    mb = np.ascontiguousarray(mb.reshape(B, KTILES, 128).transpose(0, 2, 1))
    biases = np.empty((B, 128, KTILES, 2), dtype=np.float32)
    biases[..., 0] = mb                            # ACT path: exp(scale*s + bias)
    biases[..., 1] = mb * float(1024.0 * LOG2E) + EXP_B0  # DVE Schraudolph path

    # [B, S, H, D] -> [B, H, D, S] fp16
    qt = np.ascontiguousarray(q.transpose(0, 2, 3, 1).astype(np.float16))
    kt = np.ascontiguousarray(k.transpose(0, 2, 3, 1).astype(np.float16))
    # V_aug: [B, H, 128(p), KTILES(t), VW] fp16 with ones in column D
    va = np.zeros((B, H, 128, KTILES, VW), dtype=np.float16)
    # v[b, s, h, d] with s = 128*t + p
    va[:, :, :, :, 0:D] = (
        v.reshape(B, KTILES, 128, H, D).transpose(0, 3, 2, 1, 4).astype(np.float16)
    )
    va[:, :, :, :, D] = 1.0

    in_maps = []
    for c in range(NCORES):
        sl = slice(c * H2, (c + 1) * H2)
        in_maps.append(
            {
                "qt": np.ascontiguousarray(qt[:, sl]),
                "kt": np.ascontiguousarray(kt[:, sl]),
                "va": np.ascontiguousarray(va[:, sl]),
                "biases": biases,
            }
        )
    return in_maps


def assemble_output(results):
    """Concatenate per-core [B, S, H2, D] outputs along the head axis."""
    return np.concatenate([results[c]["o"] for c in range(NCORES)], axis=2)


def kernel(q, k, v, key_padding_mask):
    from concourse.bass_utils import run_bass_kernel_spmd

    nc = _get_program()
    in_maps = make_core_inputs(q, k, v, key_padding_mask)
    res = run_bass_kernel_spmd(nc, in_maps, list(range(NCORES)))
    return assemble_output(res.results)


# revision 48
# speedup vs baseline: 1.0058x; 1.0058x over previous
"""Trainium2 Bass kernel: standard multi-head attention (B=2, S=2048, H=16, D=128, fp32).

Sharding: head-parallel across 8 NeuronCores (2 heads per core, both batches),
zero cross-core communication.

Host side (part of sharding): per core, Q and K head-slices are transposed to
[d, s] and cast to fp16; V is cast to fp16 and augmented with a ones column
(so the PV matmul accumulates the softmax denominator for free); the key
padding mask becomes additive biases laid out per k-tile (one set for the ACT
exp path, one pre-folded into the Schraudolph constant for the DVE path).

Per-core device program, per (batch, head) unit:
  - DMA qT, kT, V_aug (fp16) straight into SBUF.
  - For each 1024-wide q-chunk, 16 steps (one per k-tile):
      S^T[k, q] = (K Q^T) via PE matmul (fp16 in, fp32 PSUM).
      P^T = exp(scale * S^T + bias) -> fp16 SBUF, split across two engines:
        * 11/16 k-tiles on ACT (table exp),
        * 5/16 k-tiles on DVE via a one-instruction Schraudolph exp:
          int16 bits = round(S^T * (1024*log2e*scale) + (15360 - C + bias'))
          bit-cast to fp16. C=60 centers the ripple (~1.7% rms), keeping
          total output error ~9e-3, well under the 2e-2 gate.
    PV for the previous chunk is interleaved: per q-tile a single 16-matmul
    PSUM accumulation group over all k-tiles (d'=128 is the denominator
    column), then one DVE reciprocal + tensor_scalar multiply normalizes
    straight out of PSUM into the chunk's output staging tile.
  - One output DMA per chunk.

PE p-state is pre-warmed with dummy matmuls during the initial DMA fill so
real matmuls start at full clock.  All accumulation fp32; matmul inputs fp16.
"""

import numpy as np

B, S, H, D = 2, 2048, 16, 128
NCORES = 8
H2 = H // NCORES          # heads per core
KTILES = S // 128         # 16
VW = D + 2                # V_aug row width: 128 d + ones col + pad
QCHUNK = 1024
NQC = S // QCHUNK         # 2
QT = QCHUNK // 128        # 8 q-tiles per chunk
SCALE = 1.0 / float(np.sqrt(D))
MASK_NEG = -30.0

# Schraudolph fp16 exp constants (DVE path)
LOG2E = 1.4426950408889634
EXP_A = float(1024.0 * LOG2E * SCALE)   # multiplies the raw (unscaled) score
EXP_C = 60.0                            # ripple-centering correction
EXP_B0 = float(15360.0 - EXP_C)         # fp16 exponent-bias term
DVE_STEPS = (2, 5, 8, 11, 14)           # k-tiles handled by the DVE exp
# The pipeline ramp-in runs narrower q-chunks (unit 0: 512/512/1024) so the
# exp drain keeps pace with QK while no/little PV work is interleaved yet;
# those chunks also give DVE a larger share of the exp (GpSimd can't read
# PSUM, so only two engines are available).
DVE_STEPS_8 = (1, 3, 5, 7, 9, 11, 13, 15)
DVE_STEPS_7 = (1, 3, 5, 7, 9, 11, 13)
CHUNK_DVE = {0: DVE_STEPS_8, 1: DVE_STEPS_7, 2: DVE_STEPS_7}

N_WARM = 40                             # PE p-state warm-up matmuls

_CACHE = {}


def _build_program():
    from contextlib import ExitStack

    import concourse.tile as tile
    from concourse import bacc, mybir

    f32 = mybir.dt.float32
    f16 = mybir.dt.float16
    i16 = mybir.dt.int16

    nc = bacc.Bacc("TRN2", target_bir_lowering=False, debug=False, num_devices=NCORES)
    qt_d = nc.dram_tensor("qt", [B, H2, D, S], f16, kind="ExternalInput").ap()
    kt_d = nc.dram_tensor("kt", [B, H2, D, S], f16, kind="ExternalInput").ap()
    va_d = nc.dram_tensor("va", [B, H2, 128, KTILES, VW], f16, kind="ExternalInput").ap()
    # biases[..., 0] = ACT exp mask bias; biases[..., 1] = DVE Schraudolph const
    biases_d = nc.dram_tensor("biases", [B, 128, KTILES, 2], f32, kind="ExternalInput").ap()
    o_d = nc.dram_tensor("o", [B, S, H2, D], f32, kind="ExternalOutput").ap()

    EXP = mybir.ActivationFunctionType.Exp
    MULT = mybir.AluOpType.mult
    ADD = mybir.AluOpType.add

    with tile.TileContext(nc) as tc, ExitStack() as ctx:
        tpool = ctx.enter_context(tc.tile_pool(name="tpool", bufs=2))
        vpool = ctx.enter_context(tc.tile_pool(name="vpool", bufs=2))
        bpool = ctx.enter_context(tc.tile_pool(name="bpool", bufs=1))
        ppool = ctx.enter_context(tc.tile_pool(name="ppool", bufs=34))
        opool = ctx.enter_context(tc.tile_pool(name="opool", bufs=2))
        rpool = ctx.enter_context(tc.tile_pool(name="rpool", bufs=4))
        wpool = ctx.enter_context(tc.tile_pool(name="wpool", bufs=1))
        st_ps = ctx.enter_context(tc.tile_pool(name="st_ps", bufs=3, space="PSUM"))
        o_ps = ctx.enter_context(tc.tile_pool(name="o_ps", bufs=2, space="PSUM"))

        # mask biases for both batches, [B, 128, KTILES, 2] (partition-major)
        bias_sb = bpool.tile([128, B, KTILES, 2], f32, name="bias_sb", tag="bias")

        units = [(b, h) for b in range(B) for h in range(H2)]
        # (unit, q0, width): unit 0 ramps in with two 512-wide chunks
        chunks = [(0, 0, 512), (0, 512, 512), (0, 1024, 1024)]
        for u in range(1, len(units)):
            for qc in range(NQC):
                chunks.append((u, qc * QCHUNK, QCHUNK))

        def prep(u, fine=False):
            """DMA the unit's pre-transposed fp16 tensors into SBUF.

            fine=True (first unit only) orders/splits the loads so the first
            QK matmul's operands land as early as possible."""
            b, h = units[u]
            qt = tpool.tile([128, S], f16, name="qt_sb", tag="qt")
            kt = tpool.tile([128, S], f16, name="kt_sb", tag="kt")
            va = vpool.tile([128, KTILES, VW], f16, name="va_sb", tag="va")
            if fine:
                # overlap the critical first loads across the two HWDGE
                # queues: sync's FIFO leads with qt-lo, scalar's with kt-lo
                nc.sync.dma_start(qt[:, 0:512], qt_d[b, h, :, 0:512])
                nc.sync.dma_start(qt[:, 512:QCHUNK], qt_d[b, h, :, 512:QCHUNK])
                nc.scalar.dma_start(bias_sb[:], biases_d.rearrange("b p t e -> p b t e"))
                nc.scalar.dma_start(kt[:, 0:QCHUNK], kt_d[b, h, :, 0:QCHUNK])
                nc.sync.dma_start(kt[:, QCHUNK:S], kt_d[b, h, :, QCHUNK:S])
                nc.scalar.dma_start(va[:, 0 : KTILES // 2, :], va_d[b, h, :, 0 : KTILES // 2, :])
                nc.sync.dma_start(va[:, KTILES // 2 :, :], va_d[b, h, :, KTILES // 2 :, :])
                nc.sync.dma_start(qt[:, QCHUNK:S], qt_d[b, h, :, QCHUNK:S])
            else:
                nc.sync.dma_start(kt[:], kt_d[b, h])
                nc.sync.dma_start(qt[:], qt_d[b, h])
                nc.scalar.dma_start(va[:], va_d[b, h])
            return {"q": qt, "k": kt, "v": va}

        unit_tiles = {0: prep(0, fine=True)}

        # warm up the ACT exp table before any data arrives (table load
        # ~1.3us); emitted after prep so the scalar queue's DMAs issue first
        warm = rpool.tile([128, 1], f32, name="warm", tag="warm")
        nc.vector.memset(warm[:], 0.0)
        nc.scalar.activation(warm[:], warm[:], EXP, bias=0.0, scale=1.0)

        # PE p-state warm-up: dummy matmuls on zeroed SBUF during DMA fill.
        wz = wpool.tile([128, 512], f16, name="wz", tag="wz")
        nc.vector.memset(wz[:], 0.0)
        wacc = o_ps.tile([128, D + 1], f32, name="wacc", tag="oacc")
        for _ in range(N_WARM):
            nc.tensor.matmul(
                wacc[:], lhsT=wz[:, 0:128], rhs=wz[:, 0 : D + 1],
                start=True, stop=True,
            )

        state = {}

        def emit_s(c, j):
            """QK^T matmuls for k-tile j, then exp on ACT or DVE."""
            u, q0, w = chunks[c]
            b, h = units[u]
            tl = unit_tiles[u]
            st = st_ps.tile([128, QCHUNK], f32, name="st", tag="st")
            for half in range(w // 512):
                nc.tensor.matmul(
                    st[:, half * 512 : (half + 1) * 512],
                    lhsT=tl["k"][:, j * 128 : (j + 1) * 128],
                    rhs=tl["q"][:, q0 + half * 512 : q0 + (half + 1) * 512],
                    start=True,
                    stop=True,
                )
            pt = ppool.tile([128, QCHUNK], f16, name="pt", tag="pt")
            dve_steps = CHUNK_DVE.get(c, DVE_STEPS)
            if j in dve_steps:
                nc.vector.tensor_scalar(
                    pt[:, 0:w].bitcast(mybir.dt.int16),
                    st[:, 0:w],
                    EXP_A,
                    bias_sb[:, b, j, 1:2],
                    MULT,
                    ADD,
                )
            else:
                nc.scalar.activation(
                    pt[:, 0:w], st[:, 0:w], EXP,
                    bias=bias_sb[:, b, j, 0:1], scale=SCALE,
                )
            state[c]["pt"].append(pt)

        def emit_pv_step(c, step):
            """One step's slice of the previous chunk's PV accumulation.

            A chunk of width w has w/128 q-tiles; each q-tile's 16-matmul
            PSUM group is spread over 16/(w/128) consecutive steps."""
            stt = state[c]
            u, q0, w = chunks[c]
            qtc = w // 128                  # q-tiles in this chunk
            spq = KTILES // qtc             # steps per q-tile group
            kper = KTILES // spq            # PV matmuls emitted per step
            t, phase = step // spq, step % spq
            if phase == 0:
                stt["oacc"] = o_ps.tile([128, D + 1], f32, name="oacc", tag="oacc")
            oacc = stt["oacc"]
            for j in range(phase * kper, (phase + 1) * kper):
                nc.tensor.matmul(
                    oacc[:],
                    lhsT=stt["pt"][j][:, t * 128 : (t + 1) * 128],
                    rhs=stt["v16"][:, j, 0 : D + 1],
                    start=(j == 0),
                    stop=(j == KTILES - 1),
                )
            if phase == spq - 1:
                rec = rpool.tile([128, 1], f32, name="rec", tag="rec")
                nc.vector.reciprocal(rec[:], oacc[:, D : D + 1])
                nc.vector.tensor_scalar_mul(stt["osb"][:, t, :], oacc[:, 0:D], rec[:])
                b, h = units[u]
                t0 = q0 // 128              # global q-tile index of tile 0
                o_tiles = o_d[b, :, h, :].rearrange(
                    "(t p) d -> p t d", t=S // 128, p=128
                )
                if c == nchunks - 1 and t >= qtc // 2:
                    # last chunk's final half goes out per-q-tile from the
                    # scalar queue to shorten the post-drain critical path
                    nc.scalar.dma_start(
                        o_tiles[:, t0 + t : t0 + t + 1, :],
                        stt["osb"][:, t : t + 1, :],
                    )
                    if t == qtc - 1:
                        del state[c]
                elif t == qtc // 2 - 1 or t == qtc - 1:
                    hq = 0 if t < qtc // 2 else 1
                    sl = slice(hq * (qtc // 2), (hq + 1) * (qtc // 2))
                    nc.sync.dma_start(
                        o_tiles[:, t0 + sl.start : t0 + sl.stop, :],
                        stt["osb"][:, sl, :],
                    )
                    if t == qtc - 1:
                        del state[c]

        nchunks = len(chunks)
        for c in range(nchunks + 1):
            if c < nchunks:
                u, q0, w = chunks[c]
                state[c] = {
                    "pt": [],
                    "v16": unit_tiles[u]["v"],
                    "osb": opool.tile([128, QT, D], f32, name="osb", tag="osb"),
                }
                # prefetch next unit's tensors one chunk ahead
                is_last_of_unit = c + 1 == nchunks or chunks[c + 1][0] != u
                if is_last_of_unit and u + 1 < len(units):
                    unit_tiles[u + 1] = prep(u + 1)
            for step in range(KTILES):
                if c < nchunks:
                    emit_s(c, step)
                if c > 0:
                    emit_pv_step(c - 1, step)
            if c == nchunks:
                break

    nc.compile()
    return nc


def _get_program():
    if "nc" not in _CACHE:
        _CACHE["nc"] = _build_program()
    return _CACHE["nc"]


def make_core_inputs(q, k, v, key_padding_mask):
    """Shard full inputs into per-core input maps (host side).

    Layout work done here (part of sharding): head-slice, transpose Q/K to
    [d, s], cast to fp16, build ones-augmented V, mask -> additive biases.
    """
    q = np.asarray(q, dtype=np.float32)
    k = np.asarray(k, dtype=np.float32)
    v = np.asarray(v, dtype=np.float32)

    mb = np.where(key_padding_mask, 0.0, MASK_NEG).astype(np.float32)
    # mb[b, s] with s = 128*t + p  ->  [B, 128(p), KTILES(t)]
    mb = np.ascontiguousarray(mb.reshape(B, KTILES, 128).transpose(0, 2, 1))
    biases = np.empty((B, 128, KTILES, 2), dtype=np.float32)
    biases[..., 0] = mb                            # ACT path: exp(scale*s + bias)
    biases[..., 1] = mb * float(1024.0 * LOG2E) + EXP_B0  # DVE Schraudolph path

    # [B, S, H, D] -> [B, H, D, S] fp16
    qt = np.ascontiguousarray(q.transpose(0, 2, 3, 1).astype(np.float16))
    kt = np.ascontiguousarray(k.transpose(0, 2, 3, 1).astype(np.float16))
    # V_aug: [B, H, 128(p), KTILES(t), VW] fp16 with ones in column D
    va = np.zeros((B, H, 128, KTILES, VW), dtype=np.float16)
    # v[b, s, h, d] with s = 128*t + p
    va[:, :, :, :, 0:D] = (
        v.reshape(B, KTILES, 128, H, D).transpose(0, 3, 2, 1, 4).astype(np.float16)
    )
    va[:, :, :, :, D] = 1.0

    in_maps = []
    for c in range(NCORES):
        sl = slice(c * H2, (c + 1) * H2)
        in_maps.append(
            {
                "qt": np.ascontiguousarray(qt[:, sl]),
                "kt": np.ascontiguousarray(kt[:, sl]),
                "va": np.ascontiguousarray(va[:, sl]),
                "biases": biases,
            }
        )
    return in_maps


def assemble_output(results):
    """Concatenate per-core [B, S, H2, D] outputs along the head axis."""
    return np.concatenate([results[c]["o"] for c in range(NCORES)], axis=2)


def kernel(q, k, v, key_padding_mask):
    from concourse.bass_utils import run_bass_kernel_spmd

    nc = _get_program()
    in_maps = make_core_inputs(q, k, v, key_padding_mask)
    res = run_bass_kernel_spmd(nc, in_maps, list(range(NCORES)))
    return assemble_output(res.results)


# revision 49
# speedup vs baseline: 1.0197x; 1.0138x over previous
"""Trainium2 Bass kernel: standard multi-head attention (B=2, S=2048, H=16, D=128, fp32).

Sharding: head-parallel across 8 NeuronCores (2 heads per core, both batches),
zero cross-core communication.

Host side (part of sharding): per core, Q and K head-slices are transposed to
[d, s] and cast to fp16; V is cast to fp16 and augmented with a ones column
(so the PV matmul accumulates the softmax denominator for free); the key
padding mask becomes additive biases laid out per k-tile (one set for the ACT
exp path, one pre-folded into the Schraudolph constant for the DVE path).

Per-core device program, per (batch, head) unit:
  - DMA qT, kT, V_aug (fp16) straight into SBUF.
  - For each 1024-wide q-chunk, 16 steps (one per k-tile):
      S^T[k, q] = (K Q^T) via PE matmul (fp16 in, fp32 PSUM).
      P^T = exp(scale * S^T + bias) -> fp16 SBUF, split across two engines:
        * 11/16 k-tiles on ACT (table exp),
        * 5/16 k-tiles on DVE via a one-instruction Schraudolph exp:
          int16 bits = round(S^T * (1024*log2e*scale) + (15360 - C + bias'))
          bit-cast to fp16. C=60 centers the ripple (~1.7% rms), keeping
          total output error ~9e-3, well under the 2e-2 gate.
    PV for the previous chunk is interleaved: per q-tile a single 16-matmul
    PSUM accumulation group over all k-tiles (d'=128 is the denominator
    column), then one DVE reciprocal + tensor_scalar multiply normalizes
    straight out of PSUM into the chunk's output staging tile.
  - One output DMA per chunk.

PE p-state is pre-warmed with dummy matmuls during the initial DMA fill so
real matmuls start at full clock.  All accumulation fp32; matmul inputs fp16.
"""

import numpy as np

B, S, H, D = 2, 2048, 16, 128
NCORES = 8
H2 = H // NCORES          # heads per core
KTILES = S // 128         # 16
VW = D + 2                # V_aug row width: 128 d + ones col + pad
QCHUNK = 1024
NQC = S // QCHUNK         # 2
QT = QCHUNK // 128        # 8 q-tiles per chunk
SCALE = 1.0 / float(np.sqrt(D))
MASK_NEG = -30.0

# Schraudolph fp16 exp constants (DVE path)
LOG2E = 1.4426950408889634
EXP_A = float(1024.0 * LOG2E * SCALE)   # multiplies the raw (unscaled) score
EXP_C = 60.0                            # ripple-centering correction
EXP_B0 = float(15360.0 - EXP_C)         # fp16 exponent-bias term
DVE_STEPS = (2, 5, 8, 11, 14)           # k-tiles handled by the DVE exp
# First chunk has no PV interleave, so QK outpaces the exp drain; split the
# exp 8/8 between ACT and DVE there (GpSimd can't read PSUM).
DVE_STEPS_C0 = (1, 3, 5, 7, 9, 11, 13, 15)
POOL_STEPS_C0 = ()

N_WARM = 40                             # PE p-state warm-up matmuls

_CACHE = {}


def _build_program():
    from contextlib import ExitStack

    import concourse.tile as tile
    from concourse import bacc, mybir

    f32 = mybir.dt.float32
    f16 = mybir.dt.float16
    i16 = mybir.dt.int16

    nc = bacc.Bacc("TRN2", target_bir_lowering=False, debug=False, num_devices=NCORES)
    qt_d = nc.dram_tensor("qt", [B, H2, D, S], f16, kind="ExternalInput").ap()
    kt_d = nc.dram_tensor("kt", [B, H2, D, S], f16, kind="ExternalInput").ap()
    va_d = nc.dram_tensor("va", [B, H2, 128, KTILES, VW], f16, kind="ExternalInput").ap()
    # biases[..., 0] = ACT exp mask bias; biases[..., 1] = DVE Schraudolph const
    biases_d = nc.dram_tensor("biases", [B, 128, KTILES, 2], f32, kind="ExternalInput").ap()
    o_d = nc.dram_tensor("o", [B, S, H2, D], f32, kind="ExternalOutput").ap()

    EXP = mybir.ActivationFunctionType.Exp
    MULT = mybir.AluOpType.mult
    ADD = mybir.AluOpType.add

    with tile.TileContext(nc) as tc, ExitStack() as ctx:
        tpool = ctx.enter_context(tc.tile_pool(name="tpool", bufs=2))
        vpool = ctx.enter_context(tc.tile_pool(name="vpool", bufs=2))
        bpool = ctx.enter_context(tc.tile_pool(name="bpool", bufs=1))
        ppool = ctx.enter_context(tc.tile_pool(name="ppool", bufs=34))
        opool = ctx.enter_context(tc.tile_pool(name="opool", bufs=2))
        rpool = ctx.enter_context(tc.tile_pool(name="rpool", bufs=4))
        wpool = ctx.enter_context(tc.tile_pool(name="wpool", bufs=1))
        st_ps = ctx.enter_context(tc.tile_pool(name="st_ps", bufs=3, space="PSUM"))
        o_ps = ctx.enter_context(tc.tile_pool(name="o_ps", bufs=2, space="PSUM"))

        # mask biases for both batches, [B, 128, KTILES, 2] (partition-major)
        bias_sb = bpool.tile([128, B, KTILES, 2], f32, name="bias_sb", tag="bias")

        units = [(b, h) for b in range(B) for h in range(H2)]
        chunks = [(u, qc) for u in range(len(units)) for qc in range(NQC)]

        def prep(u, fine=False):
            """DMA the unit's pre-transposed fp16 tensors into SBUF.

            fine=True (first unit only) orders/splits the loads so the first
            QK matmul's operands land as early as possible."""
            b, h = units[u]
            qt = tpool.tile([128, S], f16, name="qt_sb", tag="qt")
            kt = tpool.tile([128, S], f16, name="kt_sb", tag="kt")
            va = vpool.tile([128, KTILES, VW], f16, name="va_sb", tag="va")
            if fine:
                # overlap the critical first loads across the two HWDGE
                # queues: sync's FIFO leads with qt-lo, scalar's with kt-lo
                nc.sync.dma_start(qt[:, 0:512], qt_d[b, h, :, 0:512])
                nc.sync.dma_start(qt[:, 512:QCHUNK], qt_d[b, h, :, 512:QCHUNK])
                nc.scalar.dma_start(bias_sb[:], biases_d.rearrange("b p t e -> p b t e"))
                nc.scalar.dma_start(kt[:, 0:QCHUNK], kt_d[b, h, :, 0:QCHUNK])
                nc.sync.dma_start(kt[:, QCHUNK:S], kt_d[b, h, :, QCHUNK:S])
                nc.scalar.dma_start(va[:, 0 : KTILES // 2, :], va_d[b, h, :, 0 : KTILES // 2, :])
                nc.sync.dma_start(va[:, KTILES // 2 :, :], va_d[b, h, :, KTILES // 2 :, :])
                nc.sync.dma_start(qt[:, QCHUNK:S], qt_d[b, h, :, QCHUNK:S])
            else:
                nc.sync.dma_start(kt[:], kt_d[b, h])
                nc.sync.dma_start(qt[:], qt_d[b, h])
                nc.scalar.dma_start(va[:], va_d[b, h])
            return {"q": qt, "k": kt, "v": va}

        unit_tiles = {0: prep(0, fine=True)}

        # warm up the ACT exp table before any data arrives (table load
        # ~1.3us); emitted after prep so the scalar queue's DMAs issue first
        warm = rpool.tile([128, 1], f32, name="warm", tag="warm")
        nc.vector.memset(warm[:], 0.0)
        nc.scalar.activation(warm[:], warm[:], EXP, bias=0.0, scale=1.0)

        # PE p-state warm-up: dummy matmuls on zeroed SBUF during DMA fill.
        wz = wpool.tile([128, 512], f16, name="wz", tag="wz")
        nc.vector.memset(wz[:], 0.0)
        wacc = o_ps.tile([128, D + 1], f32, name="wacc", tag="oacc")
        for _ in range(N_WARM):
            nc.tensor.matmul(
                wacc[:], lhsT=wz[:, 0:128], rhs=wz[:, 0 : D + 1],
                start=True, stop=True,
            )

        state = {}

        def emit_s(c, j):
            """QK^T matmuls for k-tile j, then exp on ACT or DVE."""
            u, qc = chunks[c]
            b, h = units[u]
            tl = unit_tiles[u]
            q0 = qc * QCHUNK
            st = st_ps.tile([128, QCHUNK], f32, name="st", tag="st")
            for half in range(QCHUNK // 512):
                nc.tensor.matmul(
                    st[:, half * 512 : (half + 1) * 512],
                    lhsT=tl["k"][:, j * 128 : (j + 1) * 128],
                    rhs=tl["q"][:, q0 + half * 512 : q0 + (half + 1) * 512],
                    start=True,
                    stop=True,
                )
            pt = ppool.tile([128, QCHUNK], f16, name="pt", tag="pt")
            dve_steps = DVE_STEPS_C0 if c == 0 else DVE_STEPS
            pool_steps = POOL_STEPS_C0 if c == 0 else ()
            if j in dve_steps or j in pool_steps:
                eng = nc.vector if j in dve_steps else nc.gpsimd
                eng.tensor_scalar(
                    pt[:].bitcast(mybir.dt.int16),
                    st[:],
                    EXP_A,
                    bias_sb[:, b, j, 1:2],
                    MULT,
                    ADD,
                )
            else:
                nc.scalar.activation(
                    pt[:], st[:], EXP, bias=bias_sb[:, b, j, 0:1], scale=SCALE
                )
            state[c]["pt"].append(pt)

        KH = KTILES // 2

        def emit_pv_step(c, step):
            """8 PV matmuls (half a q-tile's accumulation group) + finish."""
            stt = state[c]
            t, half = step // 2, step % 2
            if half == 0:
                stt["oacc"] = o_ps.tile([128, D + 1], f32, name="oacc", tag="oacc")
            oacc = stt["oacc"]
            for j in range(half * KH, half * KH + KH):
                nc.tensor.matmul(
                    oacc[:],
                    lhsT=stt["pt"][j][:, t * 128 : (t + 1) * 128],
                    rhs=stt["v16"][:, j, 0 : D + 1],
                    start=(j == 0),
                    stop=(j == KTILES - 1),
                )
            if half == 1:
                rec = rpool.tile([128, 1], f32, name="rec", tag="rec")
                nc.vector.reciprocal(rec[:], oacc[:, D : D + 1])
                nc.vector.tensor_scalar_mul(stt["osb"][:, t, :], oacc[:, 0:D], rec[:])
                # store in two half-chunk DMAs so the tail overlaps the drain;
                # the very last half goes out per-qtile from the vector engine
                # (no sync round-trip) to shorten the post-drain critical path
                u, qc = chunks[c]
                b, h = units[u]
                if c == nchunks - 1 and t >= QT // 2:
                    nc.scalar.dma_start(
                        o_d[b, :, h, :].rearrange(
                            "(cc t p) d -> cc t p d", cc=NQC, t=QT, p=128
                        )[qc, t],
                        stt["osb"][:, t, :],
                    )
                    if t == QT - 1:
                        del state[c]
                elif t == QT // 2 - 1 or t == QT - 1:
                    hq = 0 if t < QT // 2 else 1
                    sl = slice(hq * (QT // 2), (hq + 1) * (QT // 2))
                    nc.sync.dma_start(
                        o_d[b, :, h, :].rearrange(
                            "(cc hh t p) d -> cc hh p t d",
                            cc=NQC, hh=2, t=QT // 2, p=128,
                        )[qc, hq],
                        stt["osb"][:, sl, :],
                    )
                    if t == QT - 1:
                        del state[c]

        nchunks = len(chunks)
        for c in range(nchunks + 1):
            if c < nchunks:
                u, qc = chunks[c]
                state[c] = {
                    "pt": [],
                    "v16": unit_tiles[u]["v"],
                    "osb": opool.tile([128, QT, D], f32, name="osb", tag="osb"),
                }
                # prefetch next unit's tensors one chunk ahead
                if qc == NQC - 1 and u + 1 < len(units):
                    unit_tiles[u + 1] = prep(u + 1)
            for step in range(KTILES):
                if c < nchunks:
                    emit_s(c, step)
                if c > 0:
                    emit_pv_step(c - 1, step)
            if c == nchunks:
                break

    nc.compile()
    return nc


def _get_program():
    if "nc" not in _CACHE:
        _CACHE["nc"] = _build_program()
    return _CACHE["nc"]


def make_core_inputs(q, k, v, key_padding_mask):
    """Shard full inputs into per-core input maps (host side).

    Layout work done here (part of sharding): head-slice, transpose Q/K to
    [d, s], cast to fp16, build ones-augmented V, mask -> additive biases.
    """
    q = np.asarray(q, dtype=np.float32)
    k = np.asarray(k, dtype=np.float32)
    v = np.asarray(v, dtype=np.float32)

    mb = np.where(key_padding_mask, 0.0, MASK_NEG).astype(np.float32)
    # mb[b, s] with s = 128*t + p  ->  [B, 128(p), KTILES(t)]
    mb = np.ascontiguousarray(mb.reshape(B, KTILES, 128).transpose(0, 2, 1))
    biases = np.empty((B, 128, KTILES, 2), dtype=np.float32)
    biases[..., 0] = mb                            # ACT path: exp(scale*s + bias)
    biases[..., 1] = mb * float(1024.0 * LOG2E) + EXP_B0  # DVE Schraudolph path

    # [B, S, H, D] -> [B, H, D, S] fp16
    qt = np.ascontiguousarray(q.transpose(0, 2, 3, 1).astype(np.float16))
    kt = np.ascontiguousarray(k.transpose(0, 2, 3, 1).astype(np.float16))
    # V_aug: [B, H, 128(p), KTILES(t), VW] fp16 with ones in column D
    va = np.zeros((B, H, 128, KTILES, VW), dtype=np.float16)
    # v[b, s, h, d] with s = 128*t + p
    va[:, :, :, :, 0:D] = (
        v.reshape(B, KTILES, 128, H, D).transpose(0, 3, 2, 1, 4).astype(np.float16)
    )
    va[:, :, :, :, D] = 1.0

    in_maps = []
    for c in range(NCORES):
        sl = slice(c * H2, (c + 1) * H2)
        in_maps.append(
            {
                "qt": np.ascontiguousarray(qt[:, sl]),
                "kt": np.ascontiguousarray(kt[:, sl]),
                "va": np.ascontiguousarray(va[:, sl]),
                "biases": biases,
            }
        )
    return in_maps


def assemble_output(results):
    """Concatenate per-core [B, S, H2, D] outputs along the head axis."""
    return np.concatenate([results[c]["o"] for c in range(NCORES)], axis=2)


def kernel(q, k, v, key_padding_mask):
    from concourse.bass_utils import run_bass_kernel_spmd

    nc = _get_program()
    in_maps = make_core_inputs(q, k, v, key_padding_mask)
    res = run_bass_kernel_spmd(nc, in_maps, list(range(NCORES)))
    return assemble_output(res.results)
